# revision 18
# baseline (speedup 1.0000x reference)
"""Trainium2 Bass kernel for the 8-step attentive LSTM ("read-process" / matching
networks FLayer): B=32, T=128, E=1024, N*k=320 support vectors, K_STEPS=8.

Sharding: data-parallel over B across 8 NeuronCores (4 episodes/core), LSTM
weights replicated, the sequential K loop stays local per core. No collectives.

Per-core device program (all tensors E-major, i.e. transposed [E, rows]):
  - X_proj = x @ W_ih.T + b  is precomputed once into DRAM (bf16, gate-major
    tiles), since it is invariant across the 8 steps.
  - per step: gates^T = X_proj + W_hh.T.T @ h_in^T via PSUM accumulation
    (X_proj added by an identity-matmul into the accumulation group), gate
    nonlinearities on ACT (sigmoid via tanh identity: sigmoid(x) =
    0.5*tanh(x/2)+0.5, so every transcendental lives in the single
    "exp_and_others" ACT table set), c/h updates on DVE.
  - attention: dots = q @ S^T per episode (PE), softmax along free dim
    (DVE reduce_max -> ACT exp with accum_out -> DVE reciprocal/scale),
    att transposed via PE transpose-mode, r^T = S_nk.T.T @ att^T (PE).
  - step 0 is specialized (h=c=r=0): gates are read straight out of the
    X_proj precompute PSUM, and r0 = mean of the support set (uniform
    softmax of zero logits) is folded in via a per-partition bias add.
  - step 7 skips attention (its r is never consumed).

Matmul operands are bf16 (fp32 PSUM accumulation); state (h, c) stays fp32.
"""

import os
import sys

for _p in ("/opt/trn_rl_repo", "/root/.axon_site/_ro/trn_rl_repo"):
    if os.path.isdir(_p) and _p not in sys.path:
        sys.path.insert(0, _p)

import numpy as np
import ml_dtypes

import concourse.bass as bass
import concourse.mybir as mybir
import concourse.tile as tile
from concourse import bacc
from concourse.bass_utils import run_bass_kernel_spmd
from concourse.masks import make_identity

# Problem shape (hardcoded per contract)
B, T, E, NK, KS = 32, 128, 1024, 320, 8
CORES = 8
EPC = B // CORES          # episodes per core = 4
R = EPC * T               # rows per core = 512
G = 4 * E                 # gate rows = 4096
EC = E // 128             # E chunks = 8
GT = G // 128             # gate tiles = 32

F32 = mybir.dt.float32
BF16 = mybir.dt.bfloat16
AF = mybir.ActivationFunctionType
ALU = mybir.AluOpType
AX = mybir.AxisListType
BF16_NP = ml_dtypes.bfloat16
F32R = mybir.dt.float32r

_STATE = None  # memoized (nc,) build


def _gate_groups(e):
    """(m, scale, need_gate_act_in_phaseB) for chunk e in i,g,o order for the
    step-0 path; f (never needed at step 0 since c0=0) goes first with no ACT."""
    return [
        (8 + e, 0.5, False),   # f  (xproj only at step 0)
        (0 + e, 0.5, True),    # i
        (16 + e, 1.0, True),   # g
        (24 + e, 0.5, True),   # o
    ]


def _build():
    nc = bacc.Bacc("TRN2", target_bir_lowering=False, debug=False,
                   enable_asserts=True)

    # ---- DRAM parameters (per-core shards; host preps layouts/dtypes) ----
    whh = nc.dram_tensor("whh", [E, G], BF16, kind="ExternalInput").ap()
    wih = nc.dram_tensor("wih", [E, G], F32R, kind="ExternalInput").ap()
    xtf = nc.dram_tensor("xtf", [E, R], F32R, kind="ExternalInput").ap()
    bias1 = nc.dram_tensor("bias1", [128, GT], F32, kind="ExternalInput").ap()
    bias2 = nc.dram_tensor("bias2", [128, GT], F32, kind="ExternalInput").ap()
    stf = nc.dram_tensor("stf", [EPC, E, NK], F32, kind="ExternalInput").ap()
    stb = nc.dram_tensor("stb", [EPC, E, NK], BF16, kind="ExternalInput").ap()
    snka = nc.dram_tensor("snka", [EPC, 2, 128, E], BF16, kind="ExternalInput").ap()
    snkb = nc.dram_tensor("snkb", [EPC, 64, E], BF16, kind="ExternalInput").ap()
    r0d = nc.dram_tensor("r0", [EC, 128, EPC], F32, kind="ExternalInput").ap()
    out = nc.dram_tensor("ht_out", [E, R], F32, kind="ExternalOutput").ap()
    dbg = {}
    if os.environ.get("K_DEBUG"):
        for t in range(KS):
            dbg[f"h{t}"] = nc.dram_tensor(f"dbg_h{t}", [EC, 128, R], F32,
                                          kind="ExternalOutput").ap()
            dbg[f"r{t}"] = nc.dram_tensor(f"dbg_r{t}", [EC, 128, R], F32,
                                          kind="ExternalOutput").ap()

    with tile.TileContext(nc) as tc:
        with (
            tc.tile_pool(name="res", bufs=1) as res,
            tc.tile_pool(name="pgw", bufs=5) as pgw,
            tc.tile_pool(name="pxs", bufs=2) as pxs,
            tc.tile_pool(name="pxw", bufs=1) as pxw,
            tc.tile_pool(name="pstp", bufs=2) as pstp,
            tc.tile_pool(name="pat", bufs=2) as pat,
            tc.tile_pool(name="patt", bufs=3) as patt,
            tc.tile_pool(name="pstat", bufs=8) as pstat,
            tc.tile_pool(name="ppg", bufs=2, space="PSUM") as ppg,
            tc.tile_pool(name="ppd", bufs=2, space="PSUM") as ppd,
            tc.tile_pool(name="ppt", bufs=2, space="PSUM") as ppt,
            tc.tile_pool(name="ppr", bufs=2, space="PSUM") as ppr,
            tc.tile_pool(name="pdram", bufs=1, space="DRAM") as pdram,
        ):
            xprojd = pdram.tile([GT, 128, R], BF16, tag="xprojd")
            # ---- resident tiles ----
            ident = res.tile([128, 128], BF16, tag="ident")
            xtf_sb = res.tile([128, EC, R], F32R, tag="xtf")
            b1_sb = res.tile([128, GT], F32, tag="b1")
            b2_sb = res.tile([128, GT], F32, tag="b2")
            r0_sb = res.tile([128, EC, EPC], F32, tag="r0")
            whh_sb = res.tile([128, EC, G], BF16, tag="whh")
            snka_sb = res.tile([128, EPC, 2, E], BF16, tag="snka")
            snkb_sb = res.tile([64, EPC, E], BF16, tag="snkb")
            ht = res.tile([128, EC, R], F32, tag="ht")
            ct = res.tile([128, EC, R], F32, tag="ct")
            rt = res.tile([128, EC, R], BF16, tag="rt")
            hin = res.tile([128, EC, R], BF16, tag="hin")
            hinf = res.tile([128, EC, R], F32, tag="hinf")

            make_identity(nc, ident[:, :])
            nc.sync.dma_start(out=xtf_sb[:],
                              in_=xtf.rearrange("(k p) c -> p k c", p=128))
            nc.sync.dma_start(out=b1_sb[:], in_=bias1[:])
            nc.sync.dma_start(out=b2_sb[:], in_=bias2[:])
            nc.sync.dma_start(out=r0_sb[:], in_=r0d.rearrange("k p e -> p k e"))

            def snk_lhsT(ep, c2, e):
                """lhsT slice [nk_chunk, 128] of S_nk for episode ep, nk-chunk
                c2 (0,1 full 128; 2 is the 64-row tail), E-chunk e."""
                es = slice(e * 128, (e + 1) * 128)
                if c2 < 2:
                    return snka_sb[:, ep, c2, es]
                return snkb_sb[:, ep, es]

            # ---- phase B: X_proj precompute (+ specialized step 0 gates) ----
            for e in range(EC):
                gates = {}
                for m, sc, need in _gate_groups(e):
                    xw = pxw.tile([128, EC, 128], F32R, tag="xw")
                    nc.sync.dma_start(
                        out=xw[:],
                        in_=wih.rearrange("(k p) c -> p k c", p=128)[
                            :, :, m * 128:(m + 1) * 128],
                    )
                    ps = ppg.tile([128, R], F32, tag="pg")
                    for k in range(EC):
                        nc.tensor.matmul(ps[:], xw[:, k, :], xtf_sb[:, k, :],
                                         start=(k == 0), stop=(k == EC - 1))
                    xp = pxs.tile([128, R], BF16, tag="xp")
                    nc.scalar.activation(xp[:], ps[:], AF.Identity,
                                         bias=b1_sb[:, m:m + 1])
                    nc.sync.dma_start(out=xprojd[m], in_=xp[:])
                    if need:
                        gt_ = pgw.tile([128, R], F32, tag="gw")
                        nc.scalar.activation(gt_[:], ps[:], AF.Tanh,
                                             bias=b2_sb[:, m:m + 1], scale=sc)
                        gates[m] = gt_
                t_i, t_g, t_o = gates[e], gates[16 + e], gates[24 + e]
                # step 0 elementwise for chunk e:  c = sig(i)*tanh(g),
                # h = sig(o)*tanh(c) + x,  hin1 = h + r0
                nc.vector.tensor_scalar(t_i[:], t_i[:], 0.5, 0.5,
                                        op0=ALU.mult, op1=ALU.add)
                nc.vector.tensor_mul(ct[:, e, :], t_i[:], t_g[:])
                thc = pgw.tile([128, R], F32, tag="gw")
                nc.scalar.activation(thc[:], ct[:, e, :], AF.Tanh)
                nc.vector.tensor_scalar(t_o[:], t_o[:], 0.5, 0.5,
                                        op0=ALU.mult, op1=ALU.add)
                nc.vector.tensor_mul(ht[:, e, :], t_o[:], thc[:])
                nc.vector.tensor_add(ht[:, e, :], ht[:, e, :], xtf_sb[:, e, :].bitcast(F32))
                for ep in range(EPC):
                    cs = slice(ep * 128, (ep + 1) * 128)
                    nc.vector.tensor_scalar_add(hin[:, e, cs], ht[:, e, cs],
                                                r0_sb[:, e, ep:ep + 1])
                    nc.vector.tensor_scalar_add(hinf[:, e, cs], ht[:, e, cs],
                                                r0_sb[:, e, ep:ep + 1])

            if dbg:
                for e in range(EC):
                    nc.sync.dma_start(out=dbg["h0"][e], in_=ht[:, e, :])
            # ---- bulk resident loads (needed from step 1 on) ----
            nc.sync.dma_start(out=whh_sb[:],
                              in_=whh.rearrange("(k p) c -> p k c", p=128))
            nc.sync.dma_start(out=snka_sb[:],
                              in_=snka.rearrange("ep c p e -> p ep c e"))
            nc.sync.dma_start(out=snkb_sb[:],
                              in_=snkb.rearrange("ep p e -> p ep e"))

            # ---- steps 1..7 ----
            for t in range(1, KS):
                last = (t == KS - 1)

                # attention on h_in(t) -> r(t)   (skipped for the last step)
                if not last:
                    for ep in range(EPC):
                        cs = slice(ep * 128, (ep + 1) * 128)
                        sp = pstp.tile([128, EC, NK], F32 if t == 1 else BF16,
                                       tag="stp", name="sp")
                        nc.sync.dma_start(
                            out=sp[:],
                            in_=(stf if t == 1 else stb)[ep].rearrange(
                                "(k p) c -> p k c", p=128))
                        psd = ppd.tile([128, NK], F32, tag="pd")
                        for k in range(EC):
                            if t == 1:
                                nc.tensor.matmul(psd[:], hinf[:, k, cs],
                                                 sp[:, k, :],
                                                 start=(k == 0),
                                                 stop=(k == EC - 1))
                            else:
                                nc.tensor.matmul(psd[:], hin[:, k, cs],
                                                 sp[:, k, :],
                                                 start=(k == 0),
                                                 stop=(k == EC - 1))
                        nmax = pstat.tile([128, 1], F32, tag="stat")
                        nc.vector.tensor_reduce(nmax[:], psd[:], axis=AX.X,
                                                op=ALU.max, negate=True)
                        at = pat.tile([128, NK], BF16, tag="at")
                        sm = pstat.tile([128, 1], F32, tag="stat")
                        nc.scalar.activation(at[:], psd[:], AF.Exp,
                                             bias=nmax[:, :], accum_out=sm[:, :])
                        rec = pstat.tile([128, 1], F32, tag="stat")
                        nc.vector.reciprocal(rec[:], sm[:])
                        nc.vector.tensor_scalar_mul(at[:], at[:], rec[:, :])
                        atT = []
                        for c2 in range(3):
                            w = 128 if c2 < 2 else 64
                            pt = ppt.tile([128, 128], BF16, tag="pt")
                            nc.tensor.transpose(
                                pt[:w, :], at[:, c2 * 128:c2 * 128 + w],
                                ident[:, :])
                            aT = patt.tile([128, 128], BF16, tag="atT")
                            nc.vector.tensor_copy(aT[:w, :], pt[:w, :])
                            atT.append(aT)
                        for e in range(EC):
                            psr = ppr.tile([128, 128], F32, tag="pr")
                            for c2 in range(3):
                                w = 128 if c2 < 2 else 64
                                nc.tensor.matmul(psr[:], snk_lhsT(ep, c2, e),
                                                 atT[c2][:w, :],
                                                 start=(c2 == 0), stop=(c2 == 2))
                            nc.vector.tensor_copy(rt[:, e, cs], psr[:])

                # gates + state update, chunk-major so h chunks complete early
                for e in range(EC):
                    gates = {}
                    for m in (8 + e, 0 + e, 16 + e, 24 + e):   # f, i, g, o
                        sc = 1.0 if 16 <= m < 24 else 0.5
                        xp = pxs.tile([128, R], BF16, tag="xp")
                        nc.sync.dma_start(out=xp[:], in_=xprojd[m])
                        ps = ppg.tile([128, R], F32, tag="pg")
                        nc.tensor.matmul(ps[:], ident[:, :], xp[:],
                                         start=True, stop=False)
                        for k in range(EC):
                            nc.tensor.matmul(
                                ps[:], whh_sb[:, k, m * 128:(m + 1) * 128],
                                hin[:, k, :], start=False, stop=(k == EC - 1))
                        gt_ = pgw.tile([128, R], F32, tag="gw")
                        nc.scalar.activation(gt_[:], ps[:], AF.Tanh, scale=sc)
                        gates[m] = gt_
                    t_f, t_i, t_g, t_o = (gates[8 + e], gates[e],
                                          gates[16 + e], gates[24 + e])
                    nc.vector.tensor_scalar(t_f[:], t_f[:], 0.5, 0.5,
                                            op0=ALU.mult, op1=ALU.add)
                    nc.vector.tensor_mul(ct[:, e, :], t_f[:], ct[:, e, :])
                    nc.vector.tensor_scalar(t_i[:], t_i[:], 0.5, 0.5,
                                            op0=ALU.mult, op1=ALU.add)
                    nc.vector.tensor_mul(t_i[:], t_i[:], t_g[:])
                    nc.vector.tensor_add(ct[:, e, :], ct[:, e, :], t_i[:])
                    thc = pgw.tile([128, R], F32, tag="gw")
                    nc.scalar.activation(thc[:], ct[:, e, :], AF.Tanh)
                    nc.vector.tensor_scalar(t_o[:], t_o[:], 0.5, 0.5,
                                            op0=ALU.mult, op1=ALU.add)
                    nc.vector.tensor_mul(ht[:, e, :], t_o[:], thc[:])
                    nc.vector.tensor_add(ht[:, e, :], ht[:, e, :], xtf_sb[:, e, :].bitcast(F32))
                    if dbg:
                        nc.sync.dma_start(out=dbg[f"h{t}"][e], in_=ht[:, e, :])
                        if not last:
                            dr = pgw.tile([128, R], F32, tag="gw", name="dr")
                            nc.vector.tensor_copy(dr[:], rt[:, e, :])
                            nc.sync.dma_start(out=dbg[f"r{t}"][e], in_=dr[:])

                # write h_in(t+1) only after every matmul of step t has read
                # the old h_in (single-buffered)
                if not last:
                    for e in range(EC):
                        nc.vector.tensor_add(hin[:, e, :], ht[:, e, :],
                                             rt[:, e, :])

            for e in range(EC):
                nc.sync.dma_start(out=out[e * 128:(e + 1) * 128, :],
                                  in_=ht[:, e, :])

    nc.compile()
    return nc


def _get_nc():
    global _STATE
    if _STATE is None:
        _STATE = _build()
    return _STATE


def _prep_in_maps(targets, support_embeddings, W_ih, W_hh, b_ih, b_hh):
    bf = lambda a: np.ascontiguousarray(a).astype(BF16_NP)
    whh_bf = bf(W_hh.astype(np.float32).T)
    wih_f = np.ascontiguousarray(W_ih.astype(np.float32).T)
    bias = (b_ih + b_hh).astype(np.float32)
    bias1 = np.ascontiguousarray(bias.reshape(GT, 128).T)      # [128, GT]
    scale_col = np.where((np.arange(GT) >= 16) & (np.arange(GT) < 24), 1.0, 0.5)
    bias2 = np.ascontiguousarray(bias1 * scale_col[None, :].astype(np.float32))
    in_maps = []
    for i in range(CORES):
        x = targets[EPC * i:EPC * (i + 1)].reshape(R, E).astype(np.float32)
        S = support_embeddings[EPC * i:EPC * (i + 1)].reshape(EPC, NK, E)
        S = S.astype(np.float32)
        r0 = S.mean(axis=1)                                     # [EPC, E]
        in_maps.append({
            "whh": whh_bf,
            "wih": wih_f,
            "xtf": np.ascontiguousarray(x.T, dtype=np.float32),
            "bias1": bias1,
            "bias2": np.ascontiguousarray(bias2, dtype=np.float32),
            "stf": np.ascontiguousarray(S.transpose(0, 2, 1), dtype=np.float32),
            "stb": bf(S.transpose(0, 2, 1)),
            "snka": bf(S[:, :256, :].reshape(EPC, 2, 128, E)),
            "snkb": bf(S[:, 256:, :]),
            "r0": np.ascontiguousarray(r0.T.reshape(EC, 128, EPC),
                                       dtype=np.float32),
        })
    return in_maps


def kernel(**inputs):
    nc = _get_nc()
    in_maps = _prep_in_maps(**{k: np.asarray(v) for k, v in inputs.items()})
    res = run_bass_kernel_spmd(nc, in_maps, core_ids=list(range(CORES)))
    out = np.empty((B, T, E), dtype=np.float32)
    for i in range(CORES):
        ht = res.results[i]["ht_out"]                           # [E, R]
        out[EPC * i:EPC * (i + 1)] = ht.T.reshape(EPC, T, E)
    return out


if __name__ == "__main__":
    nc = _get_nc()
    print("build+compile OK; instructions:",
          sum(len(b.instructions) for f in nc.m.functions for b in f.blocks))


# revision 41
# speedup vs baseline: 1.3996x; 1.3996x over previous
"""Trainium2 Bass kernel for the 8-step attentive LSTM ("read-process" / matching
networks FLayer): B=32, T=128, E=1024, N*k=320 support vectors, K_STEPS=8.

Sharding: data-parallel over B across 8 NeuronCores (4 episodes/core), LSTM
weights replicated, the sequential K loop stays local per core. No collectives.

Per-core device program (all tensors E-major, i.e. transposed [E, rows]):
  - X_proj = x @ W_ih.T + b is precomputed once into DRAM (bf16 tiles) with
    float32r matmuls (full fp32 data at bf16 PE speed), since it is invariant
    across the 8 steps; it streams back one tile per gate group per step.
  - per step: gates^T = X_proj + W_hh.T.T @ h_in^T via PSUM accumulation.
    The X_proj add is load-balanced: 1/4 of the tiles join the accumulation
    group via an identity-matmul on PE, the rest are added into PSUM by DVE.
    Gate nonlinearities run on ACT using the tanh form of sigmoid
    (sigmoid(x) = 0.5*tanh(x/2)+0.5) so every transcendental (tanh, exp)
    lives in the single "exp_and_others" ACT table set - no table swaps.
  - attention: dots = q @ S^T per episode (PE), softmax along the free dim
    (DVE negated reduce_max -> ACT exp with fused accum_out sum -> DVE
    reciprocal + scale), att transposed via PE transpose-mode, then
    r^T = S_nk.T.T @ att^T (PE) drains via ACT copies directly into the
    next step's h_in buffer (double-buffered; the per-chunk h add lands
    there as soon as each h chunk is ready, hiding the step boundary).
  - precision: step-1 logits are the only chaos-sensitive spot (top-gap can
    be ~0.1 while later steps saturate to one-hot with gaps ~900), so step-1
    dots run in full fp32 against f32 h(0)/S, with the rank-1 r0 term
    (ones x (r0 . S), host-exact) added by a K=1 matmul; steps 2-6 use bf16.
    The support set streams per step (f32 once for step 1, bf16 afterward).
  - step 0 is specialized (h=c=r=0): gates come straight out of the X_proj
    PSUM, r0 = support mean is applied as a per-partition bias add.
  - step 7 skips attention (its r is never consumed).
  - the W_ih stream rides the SP HWDGE queue (host-pretiled, contiguous per
    gate tile); resident loads needed from step 1 (W_hh, S_nk) ride the ACT
    HWDGE queue so they overlap phase B instead of serializing after it.

Gates/attention matmuls are bf16 (fp32 PSUM accumulation) except where noted;
state (h, c) and the x skip-connection stay fp32 (a bf16 skip-add alone costs
~1.4e-2 rel err through the softmax's chaotic amplification).
"""

import os
import sys

for _p in ("/opt/trn_rl_repo", "/root/.axon_site/_ro/trn_rl_repo"):
    if os.path.isdir(_p) and _p not in sys.path:
        sys.path.insert(0, _p)

import numpy as np
import ml_dtypes

import concourse.bass as bass
import concourse.mybir as mybir
import concourse.tile as tile
from concourse import bacc
from concourse.bass_utils import run_bass_kernel_spmd
from concourse.masks import make_identity

# Problem shape (hardcoded per contract)
B, T, E, NK, KS = 32, 128, 1024, 320, 8
CORES = 8
EPC = B // CORES          # episodes per core = 4
R = EPC * T               # rows per core = 512
G = 4 * E                 # gate rows = 4096
EC = E // 128             # E chunks = 8
GT = G // 128             # gate tiles = 32

F32 = mybir.dt.float32
BF16 = mybir.dt.bfloat16
AF = mybir.ActivationFunctionType
ALU = mybir.AluOpType
AX = mybir.AxisListType
BF16_NP = ml_dtypes.bfloat16
F32R = mybir.dt.float32r

_STATE = {}  # memoized builds keyed by n_steps


def _gate_groups(e):
    """(m, scale, need_gate_act_in_phaseB) for chunk e in i,g,o order for the
    step-0 path; f (never needed at step 0 since c0=0) goes first with no ACT."""
    return [
        (8 + e, 0.5, False),   # f  (xproj only at step 0)
        (0 + e, 0.5, True),    # i
        (16 + e, 1.0, True),   # g
        (24 + e, 0.5, True),   # o
    ]


def _build(n_steps=KS):
    nc = bacc.Bacc("TRN2", target_bir_lowering=False, debug=False,
                   enable_asserts=True)

    # ---- DRAM parameters (per-core shards; host preps layouts/dtypes) ----
    whh = nc.dram_tensor("whh", [E, G], BF16, kind="ExternalInput").ap()
    wih = nc.dram_tensor("wih", [GT, 128, EC, 128], F32R, kind="ExternalInput").ap()
    xtf = nc.dram_tensor("xtf", [E, R], F32R, kind="ExternalInput").ap()
    bias1 = nc.dram_tensor("bias1", [128, GT], F32, kind="ExternalInput").ap()
    bias2 = nc.dram_tensor("bias2", [128, GT], F32, kind="ExternalInput").ap()
    stf = nc.dram_tensor("stf", [EPC, E, NK], F32, kind="ExternalInput").ap()
    stb = nc.dram_tensor("stb", [EPC, E, NK], BF16, kind="ExternalInput").ap()
    snka = nc.dram_tensor("snka", [EPC, 2, 128, E], BF16, kind="ExternalInput").ap()
    snkb = nc.dram_tensor("snkb", [EPC, 64, E], BF16, kind="ExternalInput").ap()
    r0d = nc.dram_tensor("r0", [EC, 128, EPC], F32, kind="ExternalInput").ap()
    r0s = nc.dram_tensor("r0s", [EPC, NK], F32, kind="ExternalInput").ap()
    out = nc.dram_tensor("ht_out", [E, R], F32, kind="ExternalOutput").ap()
    dbg = {}
    if os.environ.get("K_DEBUG"):
        for t in range(KS):
            dbg[f"h{t}"] = nc.dram_tensor(f"dbg_h{t}", [EC, 128, R], F32,
                                          kind="ExternalOutput").ap()
            dbg[f"r{t}"] = nc.dram_tensor(f"dbg_r{t}", [EC, 128, R], F32,
                                          kind="ExternalOutput").ap()

    with tile.TileContext(nc) as tc:
        with (
            tc.tile_pool(name="res", bufs=1) as res,
            tc.tile_pool(name="pgw", bufs=5) as pgw,
            tc.tile_pool(name="pxs", bufs=4) as pxs,
            tc.tile_pool(name="pxw", bufs=3) as pxw,
            tc.tile_pool(name="pstp", bufs=4) as pstp,
            tc.tile_pool(name="pat", bufs=2) as pat,
            tc.tile_pool(name="patt", bufs=3) as patt,
            tc.tile_pool(name="pstat", bufs=6) as pstat,
            tc.tile_pool(name="ppg", bufs=3, space="PSUM") as ppg,
            tc.tile_pool(name="ppd", bufs=2, space="PSUM") as ppd,
            tc.tile_pool(name="ppt", bufs=3, space="PSUM") as ppt,
            tc.tile_pool(name="pdram", bufs=1, space="DRAM") as pdram,
        ):
            xprojd = pdram.tile([GT, 128, R], BF16, tag="xprojd")
            # ---- resident tiles ----
            ident = res.tile([128, 128], BF16, tag="ident")
            xtf_sb = res.tile([128, EC, R], F32R, tag="xtf")
            b1_sb = res.tile([128, GT], F32, tag="b1")
            b2_sb = res.tile([128, GT], F32, tag="b2")
            r0_sb = res.tile([128, EC, EPC], F32, tag="r0")
            whh_sb = res.tile([128, EC, G], BF16, tag="whh")
            snka_sb = res.tile([128, EPC, 2, E], BF16, tag="snka")
            snkb_sb = res.tile([64, EPC, E], BF16, tag="snkb")
            ht = res.tile([128, EC, R], F32, tag="ht")
            ct = res.tile([128, EC, R], F32, tag="ct")
            hb = [res.tile([128, EC, R], BF16, tag=f"hb{i}", name=f"hb{i}")
                  for i in range(2)]
            ones1 = res.tile([1, 128], F32, tag="ones1")
            r0s_sb = res.tile([1, EPC, NK], F32, tag="r0s")

            make_identity(nc, ident[:, :])
            nc.sync.dma_start(out=xtf_sb[:],
                              in_=xtf.rearrange("(k p) c -> p k c", p=128))
            nc.sync.dma_start(out=b1_sb[:], in_=bias1[:])
            nc.sync.dma_start(out=b2_sb[:], in_=bias2[:])
            nc.sync.dma_start(out=r0_sb[:], in_=r0d.rearrange("k p e -> p k e"))
            nc.vector.memset(ones1[:, :], 1.0)
            nc.sync.dma_start(out=r0s_sb[:], in_=r0s[:, :])

            def snk_lhsT(ep, c2, e):
                """lhsT slice [nk_chunk, 128] of S_nk for episode ep, nk-chunk
                c2 (0,1 full 128; 2 is the 64-row tail), E-chunk e."""
                es = slice(e * 128, (e + 1) * 128)
                if c2 < 2:
                    return snka_sb[:, ep, c2, es]
                return snkb_sb[:, ep, es]

            # ---- phase B: X_proj precompute (+ specialized step 0 gates) ----
            for e in range(EC):
                gates = {}
                for m, sc, need in _gate_groups(e):
                    xw = pxw.tile([128, EC, 128], F32R, tag="xw")
                    nc.sync.dma_start(out=xw[:], in_=wih[m])
                    ps = ppg.tile([128, R], F32, tag="pg")
                    for k in range(EC):
                        nc.tensor.matmul(ps[:], xw[:, k, :], xtf_sb[:, k, :],
                                         start=(k == 0), stop=(k == EC - 1))

                    xp = pxs.tile([128, R], BF16, tag="xp")
                    nc.scalar.activation(xp[:], ps[:], AF.Identity,
                                         bias=b1_sb[:, m:m + 1])
                    nc.sync.dma_start(out=xprojd[m], in_=xp[:])
                    if need:
                        gt_ = pgw.tile([128, R], F32, tag="gw")
                        nc.scalar.activation(gt_[:], ps[:], AF.Tanh,
                                             bias=b2_sb[:, m:m + 1], scale=sc)
                        gates[m] = gt_
                t_i, t_g, t_o = gates[e], gates[16 + e], gates[24 + e]
                # step 0 elementwise for chunk e:  c = sig(i)*tanh(g),
                # h = sig(o)*tanh(c) + x,  hin1 = h + r0
                nc.vector.tensor_scalar(t_i[:], t_i[:], 0.5, 0.5,
                                        op0=ALU.mult, op1=ALU.add)
                nc.vector.tensor_mul(ct[:, e, :], t_i[:], t_g[:])
                thc = pgw.tile([128, R], F32, tag="gw")
                nc.scalar.activation(thc[:], ct[:, e, :], AF.Tanh)
                nc.vector.tensor_scalar(t_o[:], t_o[:], 0.5, 0.5,
                                        op0=ALU.mult, op1=ALU.add)
                nc.vector.tensor_mul(ht[:, e, :], t_o[:], thc[:])
                nc.vector.tensor_add(ht[:, e, :], ht[:, e, :], xtf_sb[:, e, :].bitcast(F32))
                for ep in range(EPC):
                    cs = slice(ep * 128, (ep + 1) * 128)
                    nc.vector.tensor_scalar_add(hb[1][:, e, cs], ht[:, e, cs],
                                                r0_sb[:, e, ep:ep + 1])

            if dbg:
                for e in range(EC):
                    nc.sync.dma_start(out=dbg["h0"][e], in_=ht[:, e, :])
            # ---- bulk resident loads (needed from step 1 on) ----
            # Resident loads for steps >= 1 ride the ACT engine's HWDGE
            # sequencer: it is independent of the SP sequencer that issues the
            # phase-B weight stream, so these transfers fill DMA idle slots
            # during phase B instead of serializing after it.
            whh_r = whh.rearrange("(k p) c -> p k c", p=128)
            snka_r = snka.rearrange("ep c p e -> p ep c e")
            nc.scalar.dma_start(out=snka_sb[:, 0, :, :],
                                in_=snka_r[:, 0, :, :])
            nc.scalar.dma_start(out=snkb_sb[:],
                                in_=snkb.rearrange("ep p e -> p ep e"))
            for k in range(EC):
                nc.scalar.dma_start(out=whh_sb[:, k, :], in_=whh_r[:, k, :])
            for ep in range(1, EPC):
                nc.scalar.dma_start(out=snka_sb[:, ep, :, :],
                                    in_=snka_r[:, ep, :, :])

            # ---- steps 1..7 ----
            for t in range(1, n_steps):
                last = (t == n_steps - 1)
                hcur, hnext = hb[t % 2], hb[(t + 1) % 2]

                # attention on h_in(t) -> r(t)   (skipped for the last step)
                if not last:
                    for ep in range(EPC):
                        cs = slice(ep * 128, (ep + 1) * 128)
                        if t == 1:
                            half = EC // 2
                            spa = pstp.tile([128, half, NK], F32, tag="stp",
                                            name="spa")
                            spb = pstp.tile([128, half, NK], F32, tag="stp",
                                            name="spb")
                            sview = stf[ep].rearrange("(k p) c -> p k c",
                                                      p=128)
                            nc.sync.dma_start(out=spa[:], in_=sview[:, :half, :])
                            nc.sync.dma_start(out=spb[:], in_=sview[:, half:, :])
                        else:
                            sp = pstp.tile([128, EC, NK], BF16, tag="stp",
                                           name="sp")
                            nc.sync.dma_start(
                                out=sp[:],
                                in_=stb[ep].rearrange("(k p) c -> p k c", p=128))
                        psd = ppd.tile([128, NK], F32, tag="pd")
                        if t == 1:
                            # q(1) = h(0) + r0; the r0 contribution to the
                            # logits is rank-1: ones x (r0 . S), exact in f32
                            nc.tensor.matmul(psd[:], ones1[:1, :],
                                             r0s_sb[:1, ep, :],
                                             start=True, stop=False)
                        for k in range(EC):
                            if t == 1:
                                rhs = (spa[:, k, :] if k < EC // 2
                                       else spb[:, k - EC // 2, :])
                                nc.tensor.matmul(psd[:], ht[:, k, cs], rhs,
                                                 start=False,
                                                 stop=(k == EC - 1))
                            else:
                                nc.tensor.matmul(psd[:], hcur[:, k, cs],
                                                 sp[:, k, :],
                                                 start=(k == 0),
                                                 stop=(k == EC - 1))
                        nmax = pstat.tile([128, 1], F32, tag="stat")
                        nc.vector.tensor_reduce(nmax[:], psd[:], axis=AX.X,
                                                op=ALU.max, negate=True)
                        at = pat.tile([128, NK], BF16, tag="at")
                        sm = pstat.tile([128, 1], F32, tag="stat")
                        nc.scalar.activation(at[:], psd[:], AF.Exp,
                                             bias=nmax[:, :], accum_out=sm[:, :])
                        rec = pstat.tile([128, 1], F32, tag="stat")
                        nc.vector.reciprocal(rec[:], sm[:])
                        nc.vector.tensor_scalar_mul(at[:], at[:], rec[:, :])
                        atT = []
                        for c2 in range(3):
                            w = 128 if c2 < 2 else 64
                            pt = ppt.tile([128, 128], BF16, tag="pt")
                            nc.tensor.transpose(
                                pt[:w, :], at[:, c2 * 128:c2 * 128 + w],
                                ident[:, :])
                            aT = patt.tile([128, 128], BF16, tag="atT")
                            nc.vector.tensor_copy(aT[:w, :], pt[:w, :])
                            atT.append(aT)
                        for e in range(EC):
                            psr = ppt.tile([128, 128], F32, tag="pt",
                                           name="psr")
                            for c2 in range(3):
                                w = 128 if c2 < 2 else 64
                                nc.tensor.matmul(psr[:], snk_lhsT(ep, c2, e),
                                                 atT[c2][:w, :],
                                                 start=(c2 == 0), stop=(c2 == 2))
                            nc.scalar.copy(hnext[:, e, cs], psr[:])

                # gates + state update, chunk-major so h chunks complete early
                for e in range(EC):
                    gates = {}
                    for m in (8 + e, 0 + e, 16 + e, 24 + e):   # f, i, g, o
                        sc = 1.0 if 16 <= m < 24 else 0.5
                        xp = pxs.tile([128, R], BF16, tag="xp")
                        nc.sync.dma_start(out=xp[:], in_=xprojd[m])
                        ps = ppg.tile([128, R], F32, tag="pg")
                        pe_add = (m % 4 == 0)   # split X_proj add PE/DVE
                        if pe_add:
                            nc.tensor.matmul(ps[:], ident[:, :], xp[:],
                                             start=True, stop=False)
                        for k in range(EC):
                            nc.tensor.matmul(
                                ps[:], whh_sb[:, k, m * 128:(m + 1) * 128],
                                hcur[:, k, :], start=(not pe_add and k == 0),
                                stop=(k == EC - 1))
                        if not pe_add:
                            nc.vector.tensor_add(ps[:], ps[:], xp[:])
                        gt_ = pgw.tile([128, R], F32, tag="gw")
                        nc.scalar.activation(gt_[:], ps[:], AF.Tanh, scale=sc)
                        gates[m] = gt_
                    t_f, t_i, t_g, t_o = (gates[8 + e], gates[e],
                                          gates[16 + e], gates[24 + e])
                    nc.vector.tensor_scalar(t_f[:], t_f[:], 0.5, 0.5,
                                            op0=ALU.mult, op1=ALU.add)
                    nc.vector.tensor_mul(ct[:, e, :], t_f[:], ct[:, e, :])
                    nc.vector.tensor_scalar(t_i[:], t_i[:], 0.5, 0.5,
                                            op0=ALU.mult, op1=ALU.add)
                    nc.vector.tensor_mul(t_i[:], t_i[:], t_g[:])
                    nc.vector.tensor_add(ct[:, e, :], ct[:, e, :], t_i[:])
                    thc = pgw.tile([128, R], F32, tag="gw")
                    nc.scalar.activation(thc[:], ct[:, e, :], AF.Tanh)
                    nc.vector.tensor_scalar(t_o[:], t_o[:], 0.5, 0.5,
                                            op0=ALU.mult, op1=ALU.add)
                    nc.vector.tensor_mul(ht[:, e, :], t_o[:], thc[:])
                    nc.vector.tensor_add(ht[:, e, :], ht[:, e, :], xtf_sb[:, e, :].bitcast(F32))
                    if not last:
                        nc.vector.tensor_add(hnext[:, e, :], hnext[:, e, :],
                                             ht[:, e, :])
                    if dbg:
                        nc.sync.dma_start(out=dbg[f"h{t}"][e], in_=ht[:, e, :])

            for e in range(EC):
                nc.sync.dma_start(out=out[e * 128:(e + 1) * 128, :],
                                  in_=ht[:, e, :])

    nc.compile()
    return nc


def _get_nc(n_steps=KS):
    if n_steps not in _STATE:
        _STATE[n_steps] = _build(n_steps)
    return _STATE[n_steps]


def _prep_in_maps(targets, support_embeddings, W_ih, W_hh, b_ih, b_hh):
    bf = lambda a: np.ascontiguousarray(a).astype(BF16_NP)
    whh_bf = bf(W_hh.astype(np.float32).T)
    wih_f = np.ascontiguousarray(
        W_ih.astype(np.float32).reshape(GT, 128, EC, 128).transpose(0, 3, 2, 1))
    bias = (b_ih + b_hh).astype(np.float32)
    bias1 = np.ascontiguousarray(bias.reshape(GT, 128).T)      # [128, GT]
    scale_col = np.where((np.arange(GT) >= 16) & (np.arange(GT) < 24), 1.0, 0.5)
    bias2 = np.ascontiguousarray(bias1 * scale_col[None, :].astype(np.float32))
    in_maps = []
    for i in range(CORES):
        x = targets[EPC * i:EPC * (i + 1)].reshape(R, E).astype(np.float32)
        S = support_embeddings[EPC * i:EPC * (i + 1)].reshape(EPC, NK, E)
        S = S.astype(np.float32)
        r0 = S.mean(axis=1)                                     # [EPC, E]
        r0s_h = np.einsum("pe,pne->pn", r0, S).astype(np.float32)
        in_maps.append({
            "whh": whh_bf,
            "wih": wih_f,
            "xtf": np.ascontiguousarray(x.T, dtype=np.float32),
            "bias1": bias1,
            "bias2": np.ascontiguousarray(bias2, dtype=np.float32),
            "stf": np.ascontiguousarray(S.transpose(0, 2, 1), dtype=np.float32),
            "stb": bf(S.transpose(0, 2, 1)),
            "snka": bf(S[:, :256, :].reshape(EPC, 2, 128, E)),
            "snkb": bf(S[:, 256:, :]),
            "r0": np.ascontiguousarray(r0.T.reshape(EC, 128, EPC),
                                       dtype=np.float32),
            "r0s": r0s_h,
        })
    return in_maps


def kernel(**inputs):
    nc = _get_nc()
    in_maps = _prep_in_maps(**{k: np.asarray(v) for k, v in inputs.items()})
    res = run_bass_kernel_spmd(nc, in_maps, core_ids=list(range(CORES)))
    out = np.empty((B, T, E), dtype=np.float32)
    for i in range(CORES):
        ht = res.results[i]["ht_out"]                           # [E, R]
        out[EPC * i:EPC * (i + 1)] = ht.T.reshape(EPC, T, E)
    return out


if __name__ == "__main__":
    nc = _get_nc()
    print("build+compile OK; instructions:",
          sum(len(b.instructions) for f in nc.m.functions for b in f.blocks))
